# Initial kernel scaffold
#
"""DeepPoly ReLU abstract-transformer kernel for 8 TRN2 NeuronCores.

Reference semantics (elementwise over N = 16,777,216):
    x_out     = relu(x)
    neg  = upper <= 0          -> bounds (0, 0)
    pos  = lower >= 0          -> bounds (upper, upper)
    crossing   (else)          -> (lower, upper^2 / (upper - lower))

Branch-free device formulation (all f32):
    up  = relu(upper)                      # ACT, in place on u
    nl  = relu(-lower)                     # ACT
    sq  = up^2                             # ACT (Square)
    pp  = (lower >= 0)  as uint8           # DVE  is_ge
    le  = (up <= 0)     as uint8           # GPSIMD is_le  (== upper <= 0)
    d   = up + nl                          # GPSIMD, in place on nl
    r   = 1/d                              # DVE reciprocal_approx_fast, in place
    upper_out = sq * r                     # DVE, in place on sq
      neg: 0*(1/-l) = 0; pos: u^2/u = u; crossing: u^2/(u-l)
    lower_out (in place on l):
      where(le) <- 0                       # DVE copy_predicated from zeros
      where(pp) <- upper_out               # DVE copy_predicated

Sharding: pure elementwise -> split N across the 8 cores; each core sees a
[128, 16384] f32 view of its 2,097,152-element slice. No communication.
"""

import numpy as np

import concourse.bacc as bacc
import concourse.mybir as mybir
import concourse.tile as tile
from concourse import bass_utils

N_CORES = 8
N_TOTAL = 16777216
P = 128
NCOLS = N_TOTAL // N_CORES // P  # 16384
TILE_F = 1024
BUFS = 2
OUT_DMA = "scalar"
LAYOUT = "flat"

_F32 = mybir.dt.float32
_U8 = mybir.dt.uint8
_RELU = mybir.ActivationFunctionType.Relu
_SQUARE = mybir.ActivationFunctionType.Square


def build_nc(
    ncols: int = NCOLS,
    tile_f: int = TILE_F,
    bufs: int = BUFS,
    reps: int = 1,
    out_dma: str = OUT_DMA,
    layout: str = LAYOUT,
):
    """reps > 1 repeats the whole pipeline in one NEFF (benchmarking only:
    lets wall-clock deltas cancel the per-launch dispatch overhead).
    out_dma: which engine issues the three output DMAs ("sync" shares the
    input HWDGE queue; "scalar"/"vector" use that engine's own HWDGE).
    layout: "flat" = [P, ncols] DRAM tensors, tiles are column slices whose
    per-partition rows sit ncols*4 B apart; "contig" = [ntiles, P, tile_f]
    so each tile is one dense DRAM block (better HBM locality).  The host
    view is reshaped to match in run(); elementwise, so any consistent
    bijection is fine."""
    assert ncols % tile_f == 0
    ntiles = ncols // tile_f
    nc = bacc.Bacc(
        "TRN2", target_bir_lowering=False, debug=False, num_devices=N_CORES
    )
    shape = [P, ncols] if layout == "flat" else [ntiles, P, tile_f]
    x = nc.dram_tensor("x", shape, _F32, kind="ExternalInput").ap()
    lo = nc.dram_tensor("lower", shape, _F32, kind="ExternalInput").ap()
    up = nc.dram_tensor("upper", shape, _F32, kind="ExternalInput").ap()
    xo = nc.dram_tensor("x_out", shape, _F32, kind="ExternalOutput").ap()
    loo = nc.dram_tensor("lower_out", shape, _F32, kind="ExternalOutput").ap()
    upo = nc.dram_tensor("upper_out", shape, _F32, kind="ExternalOutput").ap()

    def tslice(t, i):
        if layout == "flat":
            return t[:, i * tile_f : (i + 1) * tile_f]
        return t[i]

    with tile.TileContext(nc) as tc:
        with (
            tc.tile_pool(name="const", bufs=1) as cpool,
            tc.tile_pool(name="io", bufs=bufs) as pool,
        ):
            zt = cpool.tile([P, tile_f], _F32, tag="zero")
            nc.gpsimd.memset(zt[:], 0.0)

            def body():
                for i in range(ncols // tile_f):
                    one_iter(i)

            def one_iter(i):
                xt = pool.tile([P, tile_f], _F32, tag="x")
                lt = pool.tile([P, tile_f], _F32, tag="l")
                ut = pool.tile([P, tile_f], _F32, tag="u")
                if out_dma == "split":
                    nc.sync.dma_start(out=xt[:], in_=tslice(x, i))
                    nc.sync.dma_start(out=lt[:], in_=tslice(lo, i))
                    nc.scalar.dma_start(out=ut[:], in_=tslice(up, i))
                elif out_dma == "3q":
                    nc.gpsimd.dma_start(out=xt[:], in_=tslice(x, i))
                    nc.sync.dma_start(out=lt[:], in_=tslice(lo, i))
                    nc.sync.dma_start(out=ut[:], in_=tslice(up, i))
                else:
                    nc.sync.dma_start(out=xt[:], in_=tslice(x, i))
                    nc.sync.dma_start(out=lt[:], in_=tslice(lo, i))
                    nc.sync.dma_start(out=ut[:], in_=tslice(up, i))

                nc.scalar.activation(xt[:], xt[:], _RELU)  # x_out, in place
                nc.scalar.activation(ut[:], ut[:], _RELU)  # up = relu(u)
                nlt = pool.tile([P, tile_f], _F32, tag="nl")
                nc.scalar.activation(nlt[:], lt[:], _RELU, scale=-1.0)  # relu(-l)
                sqt = pool.tile([P, tile_f], _F32, tag="sq")
                nc.scalar.activation(sqt[:], ut[:], _SQUARE)  # up^2

                # exact masks; HW CopyPredicated requires an integer mask
                # dtype.  is_ge (not Relu(l)!) so l == 0.0 takes the pos
                # branch exactly like the reference; is_le on relu(u) is
                # exactly (upper <= 0), -0.0 included.
                ppt = pool.tile([P, tile_f], _U8, tag="pp")
                nc.vector.tensor_scalar(
                    out=ppt[:], in0=lt[:], scalar1=0.0, scalar2=None,
                    op0=mybir.AluOpType.is_ge,
                )
                let = pool.tile([P, tile_f], _U8, tag="le")
                nc.gpsimd.tensor_scalar(
                    out=let[:], in0=ut[:], scalar1=0.0, scalar2=None,
                    op0=mybir.AluOpType.is_le,
                )

                nc.gpsimd.tensor_add(out=nlt[:], in0=ut[:], in1=nlt[:])  # d
                nc.vector.reciprocal_approx_fast(out=nlt[:], in_=nlt[:])  # r
                nc.vector.tensor_mul(out=sqt[:], in0=sqt[:], in1=nlt[:])  # uo

                nc.vector.copy_predicated(out=lt[:], mask=let[:], data=zt[:])
                nc.vector.copy_predicated(out=lt[:], mask=ppt[:], data=sqt[:])

                if out_dma == "split":
                    nc.scalar.dma_start(out=tslice(xo, i), in_=xt[:])
                    nc.scalar.dma_start(out=tslice(loo, i), in_=lt[:])
                    nc.sync.dma_start(out=tslice(upo, i), in_=sqt[:])
                elif out_dma == "3q":
                    nc.gpsimd.dma_start(out=tslice(xo, i), in_=xt[:])
                    nc.scalar.dma_start(out=tslice(loo, i), in_=lt[:])
                    nc.scalar.dma_start(out=tslice(upo, i), in_=sqt[:])
                else:
                    oeng = getattr(nc, out_dma)
                    oeng.dma_start(out=tslice(xo, i), in_=xt[:])
                    oeng.dma_start(out=tslice(loo, i), in_=lt[:])
                    oeng.dma_start(out=tslice(upo, i), in_=sqt[:])

            if reps == 1:
                body()
            else:
                # benchmarking only: hardware loop keeps the body IRAM-resident
                # (a python-unrolled x32 repeat stalls on instruction fetch)
                with tc.For_i(0, reps, 1):
                    body()
    nc.compile()
    return nc


def run(inputs: dict, trace: bool = False):
    """Shard, execute on 8 cores, gather. Returns (outputs_tuple, results_obj)."""
    if LAYOUT == "flat":
        core_shape = (P, NCOLS)
    else:
        core_shape = (NCOLS // TILE_F, P, TILE_F)
    arrs = {}
    for k in ("x", "lower", "upper"):
        a = np.asarray(inputs[k], dtype=np.float32)
        arrs[k] = np.ascontiguousarray(a).reshape(N_CORES, *core_shape)
    in_maps = [
        {k: arrs[k][c] for k in ("x", "lower", "upper")} for c in range(N_CORES)
    ]
    nc = build_nc()
    res = bass_utils.run_bass_kernel_spmd(
        nc, in_maps, core_ids=list(range(N_CORES)), trace=trace
    )
    outs = []
    for name in ("x_out", "lower_out", "upper_out"):
        full = np.stack([res.results[c][name] for c in range(N_CORES)])
        outs.append(full.reshape(1, N_TOTAL).astype(np.float32, copy=False))
    return tuple(outs), res


def kernel(**inputs):
    outs, _ = run(inputs, trace=False)
    return outs



# revision 1
# speedup vs baseline: 1.0642x; 1.0642x over previous
"""DeepPoly ReLU abstract-transformer kernel for 8 TRN2 NeuronCores.

Reference semantics (elementwise over N = 16,777,216):
    x_out     = relu(x)
    neg  = upper <= 0          -> bounds (0, 0)
    pos  = lower >= 0          -> bounds (upper, upper)
    crossing   (else)          -> (lower, upper^2 / (upper - lower))

Branch-free device formulation (all f32):
    up  = relu(upper)                      # ACT, in place on u
    nl  = relu(-lower)                     # ACT
    sq  = up^2                             # ACT (Square)
    pp  = (lower >= 0)  as uint8           # DVE  is_ge
    le  = (up <= 0)     as uint8           # GPSIMD is_le  (== upper <= 0)
    d   = up + nl                          # GPSIMD, in place on nl
    r   = 1/d                              # DVE reciprocal_approx_fast, in place
    upper_out = sq * r                     # DVE, in place on sq
      neg: 0*(1/-l) = 0; pos: u^2/u = u; crossing: u^2/(u-l)
    lower_out (in place on l):
      where(le) <- 0                       # DVE copy_predicated from zeros
      where(pp) <- upper_out               # DVE copy_predicated

Sharding: pure elementwise -> split N across the 8 cores; each core sees a
[128, 16384] f32 view of its 2,097,152-element slice. No communication.
"""

import numpy as np

import concourse.bacc as bacc
import concourse.mybir as mybir
import concourse.tile as tile
from concourse import bass_utils

N_CORES = 8
N_TOTAL = 16777216
P = 128
NCOLS = N_TOTAL // N_CORES // P  # 16384
TILE_F = 1024
BUFS = 2
OUT_DMA = "scalar"
LAYOUT = "flat"

_F32 = mybir.dt.float32
_U8 = mybir.dt.uint8
_RELU = mybir.ActivationFunctionType.Relu
_SQUARE = mybir.ActivationFunctionType.Square


def build_nc(
    ncols: int = NCOLS,
    tile_f: int = TILE_F,
    bufs: int = BUFS,
    reps: int = 1,
    out_dma: str = OUT_DMA,
    layout: str = LAYOUT,
):
    """reps > 1 repeats the whole pipeline in one NEFF (benchmarking only:
    lets wall-clock deltas cancel the per-launch dispatch overhead).
    out_dma: which engine issues the three output DMAs ("sync" shares the
    input HWDGE queue; "scalar"/"vector" use that engine's own HWDGE).
    layout: "flat" = [P, ncols] DRAM tensors, tiles are column slices whose
    per-partition rows sit ncols*4 B apart; "contig" = [ntiles, P, tile_f]
    so each tile is one dense DRAM block (better HBM locality).  The host
    view is reshaped to match in run(); elementwise, so any consistent
    bijection is fine."""
    assert ncols % tile_f == 0
    ntiles = ncols // tile_f
    nc = bacc.Bacc(
        "TRN2", target_bir_lowering=False, debug=False, num_devices=N_CORES
    )
    shape = [P, ncols] if layout == "flat" else [ntiles, P, tile_f]
    x = nc.dram_tensor("x", shape, _F32, kind="ExternalInput").ap()
    lo = nc.dram_tensor("lower", shape, _F32, kind="ExternalInput").ap()
    up = nc.dram_tensor("upper", shape, _F32, kind="ExternalInput").ap()
    xo = nc.dram_tensor("x_out", shape, _F32, kind="ExternalOutput").ap()
    loo = nc.dram_tensor("lower_out", shape, _F32, kind="ExternalOutput").ap()
    upo = nc.dram_tensor("upper_out", shape, _F32, kind="ExternalOutput").ap()

    def tslice(t, i):
        if layout == "flat":
            return t[:, i * tile_f : (i + 1) * tile_f]
        return t[i]

    with tile.TileContext(nc) as tc:
        with (
            tc.tile_pool(name="const", bufs=1) as cpool,
            tc.tile_pool(name="io", bufs=bufs) as pool,
        ):
            zt = cpool.tile([P, tile_f], _F32, tag="zero")
            nc.gpsimd.memset(zt[:], 0.0)

            def body():
                for i in range(ncols // tile_f):
                    one_iter(i)

            def one_iter(i):
                xt = pool.tile([P, tile_f], _F32, tag="x")
                lt = pool.tile([P, tile_f], _F32, tag="l")
                ut = pool.tile([P, tile_f], _F32, tag="u")
                if out_dma == "split":
                    nc.sync.dma_start(out=xt[:], in_=tslice(x, i))
                    nc.sync.dma_start(out=lt[:], in_=tslice(lo, i))
                    nc.scalar.dma_start(out=ut[:], in_=tslice(up, i))
                elif out_dma == "3q":
                    nc.gpsimd.dma_start(out=xt[:], in_=tslice(x, i))
                    nc.sync.dma_start(out=lt[:], in_=tslice(lo, i))
                    nc.sync.dma_start(out=ut[:], in_=tslice(up, i))
                else:
                    nc.sync.dma_start(out=xt[:], in_=tslice(x, i))
                    nc.sync.dma_start(out=lt[:], in_=tslice(lo, i))
                    nc.sync.dma_start(out=ut[:], in_=tslice(up, i))

                nc.scalar.activation(xt[:], xt[:], _RELU)  # x_out, in place
                nc.scalar.activation(ut[:], ut[:], _RELU)  # up = relu(u)
                nlt = pool.tile([P, tile_f], _F32, tag="nl")
                nc.scalar.activation(nlt[:], lt[:], _RELU, scale=-1.0)  # relu(-l)
                sqt = pool.tile([P, tile_f], _F32, tag="sq")
                nc.scalar.activation(sqt[:], ut[:], _SQUARE)  # up^2

                # exact masks; HW CopyPredicated requires an integer mask
                # dtype.  is_ge (not Relu(l)!) so l == 0.0 takes the pos
                # branch exactly like the reference; is_le on relu(u) is
                # exactly (upper <= 0), -0.0 included.
                ppt = pool.tile([P, tile_f], _U8, tag="pp")
                nc.vector.tensor_scalar(
                    out=ppt[:], in0=lt[:], scalar1=0.0, scalar2=None,
                    op0=mybir.AluOpType.is_ge,
                )
                let = pool.tile([P, tile_f], _U8, tag="le")
                nc.gpsimd.tensor_scalar(
                    out=let[:], in0=ut[:], scalar1=0.0, scalar2=None,
                    op0=mybir.AluOpType.is_le,
                )

                nc.gpsimd.tensor_add(out=nlt[:], in0=ut[:], in1=nlt[:])  # d
                nc.vector.reciprocal_approx_fast(out=nlt[:], in_=nlt[:])  # r
                nc.vector.tensor_mul(out=sqt[:], in0=sqt[:], in1=nlt[:])  # uo

                nc.vector.copy_predicated(out=lt[:], mask=let[:], data=zt[:])
                nc.vector.copy_predicated(out=lt[:], mask=ppt[:], data=sqt[:])

                if out_dma == "split":
                    nc.scalar.dma_start(out=tslice(xo, i), in_=xt[:])
                    nc.scalar.dma_start(out=tslice(loo, i), in_=lt[:])
                    nc.sync.dma_start(out=tslice(upo, i), in_=sqt[:])
                elif out_dma == "3q":
                    nc.gpsimd.dma_start(out=tslice(xo, i), in_=xt[:])
                    nc.scalar.dma_start(out=tslice(loo, i), in_=lt[:])
                    nc.scalar.dma_start(out=tslice(upo, i), in_=sqt[:])
                else:
                    oeng = getattr(nc, out_dma)
                    oeng.dma_start(out=tslice(xo, i), in_=xt[:])
                    oeng.dma_start(out=tslice(loo, i), in_=lt[:])
                    oeng.dma_start(out=tslice(upo, i), in_=sqt[:])

            if reps == 1:
                body()
            else:
                # benchmarking only: hardware loop keeps the body IRAM-resident
                # (a python-unrolled x32 repeat stalls on instruction fetch)
                with tc.For_i(0, reps, 1):
                    body()
    nc.compile()
    return nc


def run(inputs: dict, trace: bool = False):
    """Shard, execute on 8 cores, gather. Returns (outputs_tuple, results_obj)."""
    if LAYOUT == "flat":
        core_shape = (P, NCOLS)
    else:
        core_shape = (NCOLS // TILE_F, P, TILE_F)
    arrs = {}
    for k in ("x", "lower", "upper"):
        a = np.asarray(inputs[k], dtype=np.float32)
        arrs[k] = np.ascontiguousarray(a).reshape(N_CORES, *core_shape)
    in_maps = [
        {k: arrs[k][c] for k in ("x", "lower", "upper")} for c in range(N_CORES)
    ]
    nc = build_nc()
    res = bass_utils.run_bass_kernel_spmd(
        nc, in_maps, core_ids=list(range(N_CORES)), trace=trace
    )
    outs = []
    for name in ("x_out", "lower_out", "upper_out"):
        full = np.stack([res.results[c][name] for c in range(N_CORES)])
        outs.append(full.reshape(1, N_TOTAL).astype(np.float32, copy=False))
    return tuple(outs), res


def kernel(**inputs):
    outs, _ = run(inputs, trace=False)
    return outs

